# revision 7
# baseline (speedup 1.0000x reference)
"""GarNet kernel: baseline structure + host-transposed inputs (full-bandwidth
DMA) + separate wbar matmul (no interleaved ones column).

Math (per example b):
    w    = exp(-d_av^2)                      [V=128, S=16]
    hi   = w^T @ fi_v / V                    [S, N=64]
    out  = mean_V(w)[:, None] * hi           [S, N] -> flattened [S*N]

Inputs are pre-transposed on the host: fi_T [V, bpc, N], d_T [V, bpc, S]
so every DMA has >=2KB contiguous runs (no small-descriptor penalty).
wbar arrives per 8-example psum group from one extra [128,1] matmul
(lhsT = the group's packed w block, rhs = ones/V^2).
"""

import numpy as np
from contextlib import ExitStack

import concourse.bass as bass
import concourse.tile as tile
from concourse import mybir

B, V, S, N = 4096, 128, 16, 64
NCORES = 8
BPC = B // NCORES
ONES_VAL = 1.0 / (V * V)


def split_multi_waits(nc):
    fn = nc.m.functions[0]
    for block in fn.blocks:
        insts = list(block.instructions)
        changed = False
        new = []
        for inst in insts:
            si = inst.sync_info
            waits = list(si.on_wait) if (si and si.on_wait) else []
            if len(waits) > 1:
                changed = True
                for w in waits:
                    ev = mybir.InstEventSemaphore(
                        name=nc.get_next_instruction_name(), ins=[], outs=[]
                    )
                    ev.engine = inst.engine
                    ev.sync_info = mybir.SyncInfo(on_wait=[w], on_update=[])
                    new.append(ev)
                ups = list(si.on_update) if si.on_update else []
                inst.sync_info = mybir.SyncInfo(on_wait=[], on_update=ups)
            new.append(inst)
        if changed:
            block.instructions = new


def build(bpc=BPC, name="garnet3"):
    nc = bass.Bass(name=name)
    fi = nc.dram_tensor("fi_t", (V, bpc, N), mybir.dt.float32, kind="ExternalInput")
    dav = nc.dram_tensor("d_t", (V, bpc, S), mybir.dt.float32, kind="ExternalInput")
    out = nc.dram_tensor("out", (bpc, S * N), mybir.dt.float32, kind="ExternalOutput")

    f32 = mybir.dt.float32
    with tile.TileContext(nc) as tc, ExitStack() as ctx:
        warmpool = ctx.enter_context(tc.tile_pool(name="warmpool", bufs=1))
        fipool = ctx.enter_context(tc.tile_pool(name="fipool", bufs=4))
        dpool = ctx.enter_context(tc.tile_pool(name="dpool", bufs=3))
        dcpool = ctx.enter_context(tc.tile_pool(name="dcpool", bufs=3))
        opool = ctx.enter_context(tc.tile_pool(name="opool", bufs=3))
        wscpool = ctx.enter_context(tc.tile_pool(name="wscpool", bufs=6))
        psum = ctx.enter_context(tc.tile_pool(name="psum", bufs=4, space="PSUM"))
        wqpool = ctx.enter_context(tc.tile_pool(name="wqpool", bufs=3, space="PSUM"))
        wpsum = ctx.enter_context(tc.tile_pool(name="wpsum", bufs=1, space="PSUM"))

        with tc.high_priority():
            ones = warmpool.tile([128, 1], f32)
            nc.vector.memset(ones, ONES_VAL)
            wz = warmpool.tile([128, 1], f32)
            nc.vector.memset(wz, 0.0)
            wbig = warmpool.tile([128, 1], f32)
            nc.vector.memset(wbig, -88.0)
            wz2 = warmpool.tile([128, 128], f32)
            nc.vector.memset(wz2, 0.0)
            wps = wpsum.tile([128, 128], f32)
            # dummy matmul chain keeps the PE p-state ramp warm from t~0.5us
            for _ in range(8):
                nc.tensor.matmul(
                    out=wps[0:1, :], lhsT=wz, rhs=wz2, start=True, stop=True
                )
        wzero = warmpool.tile([128, 1], f32)

        sizes = [16, 16] + [24] * ((bpc - 32) // 24)
        assert sum(sizes) == bpc
        pending_store = None
        b0 = 0
        gctr = 0
        for c, E in enumerate(sizes):
            G, Q = E // 8, E // 2
            fi_t = fipool.tile([128, E, N], f32)
            if c == 0:
                # warmup: split the first load across both queues
                for k in range(2):
                    eng = nc.sync if k == 0 else nc.gpsimd
                    eng.dma_start(
                        out=fi_t[:, 8 * k : 8 * k + 8, :],
                        in_=fi[:, b0 + 8 * k : b0 + 8 * k + 8, :],
                    )
            else:
                fi_eng = nc.sync if c % 2 == 0 else nc.gpsimd
                fi_eng.dma_start(out=fi_t, in_=fi[:, b0 : b0 + E, :])
            # d: contiguous load, then square into (w_even, 0, w_odd) slots
            d_c = dcpool.tile([128, E, S], f32)
            nc.scalar.dma_start(out=d_c, in_=dav[:, b0 : b0 + E, :])
            d_t = dpool.tile([128, Q, 3, S], f32)
            nc.vector.memset(d_t[:, :, 1, :], 0.0)
            wslots = d_t[:, :, 0:3:2, :]
            d_cv = d_c.rearrange("v (q t) s -> v q t s", t=2)
            if c == 0:
                # warm op: loads the act table while chunk 0's DMAs land and
                # produces the exact-zero bias column used by chunk 0's exp
                nc.scalar.activation(
                    wzero, wz, mybir.ActivationFunctionType.Exp,
                    scale=0.0, bias=wbig,
                )
                nc.vector.tensor_mul(wslots, d_cv, d_cv)
                nc.scalar.activation(
                    wslots, wslots, mybir.ActivationFunctionType.Exp,
                    scale=-1.0, bias=wzero,
                )
            else:
                nc.vector.tensor_mul(wslots, d_cv, d_cv)
                nc.scalar.activation(
                    wslots, wslots, mybir.ActivationFunctionType.Exp, scale=-1.0
                )

            if pending_store is not None:
                st_dst, st_src = pending_store
                nc.scalar.dma_start(out=st_dst, in_=st_src)

            o_t = opool.tile([128, G, N], f32)
            for g in range(G):
                ps = psum.tile([128, N], f32)
                wq = wqpool.tile([128, 1], f32)
                for jj in range(8):
                    e = g * 8 + jj
                    q, t = e // 2, e % 2
                    nc.tensor.matmul(
                        out=ps[32 * (jj // 2) : 32 * (jj // 2) + 32, :],
                        lhsT=d_t[:, q, t : t + 2, :],
                        rhs=fi_t[:, e, :],
                        start=(t == 0),
                        stop=(t == 1),
                        tile_position=(0, 32 * (jj // 2)),
                    )
                # wbar column in its own bank. HW weights APs must collapse
                # to one free dim, so per pair q: (even,zero) then (zero,odd)
                # slot windows accumulate into partitions 32q..32q+32.
                for qq in range(4):
                    q = 4 * g + qq
                    for tt in range(2):
                        nc.tensor.matmul(
                            out=wq[32 * qq : 32 * qq + 32, :],
                            lhsT=d_t[:, q, tt : tt + 2, :],
                            rhs=ones,
                            start=(tt == 0),
                            stop=(tt == 1),
                            tile_position=(0, 32 * qq),
                        )
                wsc = wscpool.tile([128, 1], f32)
                nc.vector.tensor_copy(wsc, wq)
                nc.vector.tensor_scalar_mul(o_t[:, g, :], ps, wsc)

            if c < len(sizes) - 1:
                dst = out[b0 : b0 + E].rearrange(
                    "(g jj) (s n) -> (jj s) g n", jj=8, s=S
                )
                pending_store = (dst, o_t)
            else:
                # drain fast: store the last chunk per group
                for g in range(G):
                    dst_g = out[b0 + 8 * g : b0 + 8 * g + 8].rearrange(
                        "jj (s n) -> (jj s) n", s=S
                    )
                    nc.scalar.dma_start(out=dst_g, in_=o_t[:, g, :])
            b0 += E

    return nc


_NC_CACHE = {}


def _get_nc():
    if "nc" not in _NC_CACHE:
        nc = build()
        split_multi_waits(nc)
        _NC_CACHE["nc"] = nc
    return _NC_CACHE["nc"]


def _host_pre(fi_v, d_av, core):
    sl = slice(core * BPC, (core + 1) * BPC)
    return {
        "fi_t": np.ascontiguousarray(fi_v[sl].transpose(1, 0, 2)),
        "d_t": np.ascontiguousarray(d_av[sl].transpose(1, 0, 2)),
    }


def kernel(fi_v: np.ndarray, d_av: np.ndarray) -> np.ndarray:
    from concourse.bass_utils import run_bass_kernel_spmd

    fi_v = np.ascontiguousarray(np.asarray(fi_v, dtype=np.float32))
    d_av = np.ascontiguousarray(np.asarray(d_av, dtype=np.float32))
    assert fi_v.shape == (B, V, N) and d_av.shape == (B, V, S)
    nc = _get_nc()
    in_maps = [_host_pre(fi_v, d_av, c) for c in range(NCORES)]
    res = run_bass_kernel_spmd(nc, in_maps, core_ids=list(range(NCORES)))
    return np.concatenate([res.results[c]["out"] for c in range(NCORES)], axis=0)


# revision 8
# speedup vs baseline: 1.0016x; 1.0016x over previous
"""GarNet kernel: baseline structure + host-transposed inputs (full-bandwidth
DMA) + separate wbar matmul (no interleaved ones column).

Math (per example b):
    w    = exp(-d_av^2)                      [V=128, S=16]
    hi   = w^T @ fi_v / V                    [S, N=64]
    out  = mean_V(w)[:, None] * hi           [S, N] -> flattened [S*N]

Inputs are pre-transposed on the host: fi_T [V, bpc, N], d_T [V, bpc, S]
so every DMA has >=2KB contiguous runs (no small-descriptor penalty).
wbar arrives per 8-example psum group from one extra [128,1] matmul
(lhsT = the group's packed w block, rhs = ones/V^2).
"""

import numpy as np
from contextlib import ExitStack

import concourse.bass as bass
import concourse.tile as tile
from concourse import mybir

B, V, S, N = 4096, 128, 16, 64
NCORES = 8
BPC = B // NCORES
ONES_VAL = 1.0 / (V * V)


def split_multi_waits(nc):
    fn = nc.m.functions[0]
    for block in fn.blocks:
        insts = list(block.instructions)
        changed = False
        new = []
        for inst in insts:
            si = inst.sync_info
            waits = list(si.on_wait) if (si and si.on_wait) else []
            if len(waits) > 1:
                changed = True
                for w in waits:
                    ev = mybir.InstEventSemaphore(
                        name=nc.get_next_instruction_name(), ins=[], outs=[]
                    )
                    ev.engine = inst.engine
                    ev.sync_info = mybir.SyncInfo(on_wait=[w], on_update=[])
                    new.append(ev)
                ups = list(si.on_update) if si.on_update else []
                inst.sync_info = mybir.SyncInfo(on_wait=[], on_update=ups)
            new.append(inst)
        if changed:
            block.instructions = new


def build(bpc=BPC, name="garnet3", taper=0, last_eng="s", psb=4, wqb=3, warm_n=8):
    nc = bass.Bass(name=name)
    fi = nc.dram_tensor("fi_t", (V, bpc, N), mybir.dt.float32, kind="ExternalInput")
    dav = nc.dram_tensor("d_t", (V, bpc, S), mybir.dt.float32, kind="ExternalInput")
    out = nc.dram_tensor("out", (bpc, S * N), mybir.dt.float32, kind="ExternalOutput")

    f32 = mybir.dt.float32
    with tile.TileContext(nc) as tc, ExitStack() as ctx:
        warmpool = ctx.enter_context(tc.tile_pool(name="warmpool", bufs=1))
        fipool = ctx.enter_context(tc.tile_pool(name="fipool", bufs=4))
        dpool = ctx.enter_context(tc.tile_pool(name="dpool", bufs=3))
        dcpool = ctx.enter_context(tc.tile_pool(name="dcpool", bufs=3))
        opool = ctx.enter_context(tc.tile_pool(name="opool", bufs=3))
        wscpool = ctx.enter_context(tc.tile_pool(name="wscpool", bufs=6))
        psum = ctx.enter_context(tc.tile_pool(name="psum", bufs=psb, space="PSUM"))
        wqpool = ctx.enter_context(tc.tile_pool(name="wqpool", bufs=wqb, space="PSUM"))
        wpsum = ctx.enter_context(tc.tile_pool(name="wpsum", bufs=1, space="PSUM"))

        with tc.high_priority():
            ones = warmpool.tile([128, 1], f32)
            nc.vector.memset(ones, ONES_VAL)
            wz = warmpool.tile([128, 1], f32)
            nc.vector.memset(wz, 0.0)
            wbig = warmpool.tile([128, 1], f32)
            nc.vector.memset(wbig, -88.0)
            wz2 = warmpool.tile([128, 128], f32)
            nc.vector.memset(wz2, 0.0)
            wps = wpsum.tile([128, 128], f32)
            # dummy matmul chain keeps the PE p-state ramp warm from t~0.5us
            for _ in range(warm_n):
                nc.tensor.matmul(
                    out=wps[0:1, :], lhsT=wz, rhs=wz2, start=True, stop=True
                )
        wzero = warmpool.tile([128, 1], f32)

        if taper == 0:
            sizes = [16, 16] + [24] * ((bpc - 32) // 24)
        elif taper == 1:
            sizes = [16, 16] + [24] * 19 + [16, 8]
        elif taper == 2:
            sizes = [16, 16] + [24] * 19 + [8, 8, 8]
        elif taper == 3:
            sizes = [16, 16] + [24] * 18 + [16, 16, 16]
        elif taper == 4:
            sizes = [8, 16] + [24] * 20 + [8]
        elif taper == 5:
            sizes = [8, 8, 16] + [24] * 20
        elif taper == 6:
            sizes = [8, 16, 24] + [24] * 19 + [8]
        assert sum(sizes) == bpc
        pending_store = None
        b0 = 0
        gctr = 0
        for c, E in enumerate(sizes):
            G, Q = E // 8, E // 2
            fi_t = fipool.tile([128, E, N], f32)
            if c == 0:
                # warmup: split the first load across both queues
                h = E // 2
                for k in range(2):
                    eng = nc.sync if k == 0 else nc.gpsimd
                    eng.dma_start(
                        out=fi_t[:, h * k : h * k + h, :],
                        in_=fi[:, b0 + h * k : b0 + h * k + h, :],
                    )
            else:
                fi_eng = nc.sync if c % 2 == 0 else nc.gpsimd
                fi_eng.dma_start(out=fi_t, in_=fi[:, b0 : b0 + E, :])
            # d: contiguous load, then square into (w_even, 0, w_odd) slots
            d_c = dcpool.tile([128, E, S], f32)
            nc.scalar.dma_start(out=d_c, in_=dav[:, b0 : b0 + E, :])
            d_t = dpool.tile([128, Q, 3, S], f32)
            nc.vector.memset(d_t[:, :, 1, :], 0.0)
            wslots = d_t[:, :, 0:3:2, :]
            d_cv = d_c.rearrange("v (q t) s -> v q t s", t=2)
            if c == 0:
                # warm op: loads the act table while chunk 0's DMAs land and
                # produces the exact-zero bias column used by chunk 0's exp
                nc.scalar.activation(
                    wzero, wz, mybir.ActivationFunctionType.Exp,
                    scale=0.0, bias=wbig,
                )
                nc.vector.tensor_mul(wslots, d_cv, d_cv)
                nc.scalar.activation(
                    wslots, wslots, mybir.ActivationFunctionType.Exp,
                    scale=-1.0, bias=wzero,
                )
            else:
                nc.vector.tensor_mul(wslots, d_cv, d_cv)
                nc.scalar.activation(
                    wslots, wslots, mybir.ActivationFunctionType.Exp, scale=-1.0
                )

            if pending_store is not None:
                st_dst, st_src = pending_store
                nc.scalar.dma_start(out=st_dst, in_=st_src)

            o_t = opool.tile([128, G, N], f32)
            for g in range(G):
                ps = psum.tile([128, N], f32)
                wq = wqpool.tile([128, 1], f32)
                for jj in range(8):
                    e = g * 8 + jj
                    q, t = e // 2, e % 2
                    nc.tensor.matmul(
                        out=ps[32 * (jj // 2) : 32 * (jj // 2) + 32, :],
                        lhsT=d_t[:, q, t : t + 2, :],
                        rhs=fi_t[:, e, :],
                        start=(t == 0),
                        stop=(t == 1),
                        tile_position=(0, 32 * (jj // 2)),
                    )
                # wbar column in its own bank. HW weights APs must collapse
                # to one free dim, so per pair q: (even,zero) then (zero,odd)
                # slot windows accumulate into partitions 32q..32q+32.
                for qq in range(4):
                    q = 4 * g + qq
                    for tt in range(2):
                        nc.tensor.matmul(
                            out=wq[32 * qq : 32 * qq + 32, :],
                            lhsT=d_t[:, q, tt : tt + 2, :],
                            rhs=ones,
                            start=(tt == 0),
                            stop=(tt == 1),
                            tile_position=(0, 32 * qq),
                        )
                wsc = wscpool.tile([128, 1], f32)
                nc.vector.tensor_copy(wsc, wq)
                nc.vector.tensor_scalar_mul(o_t[:, g, :], ps, wsc)

            if c < len(sizes) - 1:
                dst = out[b0 : b0 + E].rearrange(
                    "(g jj) (s n) -> (jj s) g n", jj=8, s=S
                )
                pending_store = (dst, o_t)
            else:
                # drain fast: store the last chunk per group
                emap = {"a": [nc.scalar], "s": [nc.sync],
                        "as": [nc.scalar, nc.sync], "asg": [nc.scalar, nc.sync, nc.gpsimd]}
                engs = emap[last_eng]
                for g in range(G):
                    dst_g = out[b0 + 8 * g : b0 + 8 * g + 8].rearrange(
                        "jj (s n) -> (jj s) n", s=S
                    )
                    engs[g % len(engs)].dma_start(out=dst_g, in_=o_t[:, g, :])
            b0 += E

    return nc


_NC_CACHE = {}


def _get_nc():
    if "nc" not in _NC_CACHE:
        nc = build()
        split_multi_waits(nc)
        _NC_CACHE["nc"] = nc
    return _NC_CACHE["nc"]


def _host_pre(fi_v, d_av, core):
    sl = slice(core * BPC, (core + 1) * BPC)
    return {
        "fi_t": np.ascontiguousarray(fi_v[sl].transpose(1, 0, 2)),
        "d_t": np.ascontiguousarray(d_av[sl].transpose(1, 0, 2)),
    }


def kernel(fi_v: np.ndarray, d_av: np.ndarray) -> np.ndarray:
    from concourse.bass_utils import run_bass_kernel_spmd

    fi_v = np.ascontiguousarray(np.asarray(fi_v, dtype=np.float32))
    d_av = np.ascontiguousarray(np.asarray(d_av, dtype=np.float32))
    assert fi_v.shape == (B, V, N) and d_av.shape == (B, V, S)
    nc = _get_nc()
    in_maps = [_host_pre(fi_v, d_av, c) for c in range(NCORES)]
    res = run_bass_kernel_spmd(nc, in_maps, core_ids=list(range(NCORES)))
    return np.concatenate([res.results[c]["out"] for c in range(NCORES)], axis=0)


# revision 9
# speedup vs baseline: 1.0047x; 1.0031x over previous
"""GarNet kernel: baseline structure + host-transposed inputs (full-bandwidth
DMA) + separate wbar matmul (no interleaved ones column).

Math (per example b):
    w    = exp(-d_av^2)                      [V=128, S=16]
    hi   = w^T @ fi_v / V                    [S, N=64]
    out  = mean_V(w)[:, None] * hi           [S, N] -> flattened [S*N]

Inputs are pre-transposed on the host: fi_T [V, bpc, N], d_T [V, bpc, S]
so every DMA has >=2KB contiguous runs (no small-descriptor penalty).
wbar arrives per 8-example psum group from one extra [128,1] matmul
(lhsT = the group's packed w block, rhs = ones/V^2).
"""

import numpy as np
from contextlib import ExitStack

import concourse.bass as bass
import concourse.tile as tile
from concourse import mybir

B, V, S, N = 4096, 128, 16, 64
NCORES = 8
BPC = B // NCORES
ONES_VAL = 1.0 / (V * V)


def split_multi_waits(nc):
    fn = nc.m.functions[0]
    for block in fn.blocks:
        insts = list(block.instructions)
        changed = False
        new = []
        for inst in insts:
            si = inst.sync_info
            waits = list(si.on_wait) if (si and si.on_wait) else []
            if len(waits) > 1:
                changed = True
                for w in waits:
                    ev = mybir.InstEventSemaphore(
                        name=nc.get_next_instruction_name(), ins=[], outs=[]
                    )
                    ev.engine = inst.engine
                    ev.sync_info = mybir.SyncInfo(on_wait=[w], on_update=[])
                    new.append(ev)
                ups = list(si.on_update) if si.on_update else []
                inst.sync_info = mybir.SyncInfo(on_wait=[], on_update=ups)
            new.append(inst)
        if changed:
            block.instructions = new


def build(bpc=BPC, name="garnet3", taper=0, last_eng="s", psb=4, wqb=3, warm_n=8, fib=4, ob=3, dcb=3, wsb=6):
    nc = bass.Bass(name=name)
    fi = nc.dram_tensor("fi_t", (V, bpc, N), mybir.dt.float32, kind="ExternalInput")
    dav = nc.dram_tensor("d_t", (V, bpc, S), mybir.dt.float32, kind="ExternalInput")
    out = nc.dram_tensor("out", (bpc, S * N), mybir.dt.float32, kind="ExternalOutput")

    f32 = mybir.dt.float32
    with tile.TileContext(nc) as tc, ExitStack() as ctx:
        warmpool = ctx.enter_context(tc.tile_pool(name="warmpool", bufs=1))
        fipool = ctx.enter_context(tc.tile_pool(name="fipool", bufs=fib))
        dpool = ctx.enter_context(tc.tile_pool(name="dpool", bufs=3))
        dcpool = ctx.enter_context(tc.tile_pool(name="dcpool", bufs=dcb))
        opool = ctx.enter_context(tc.tile_pool(name="opool", bufs=ob))
        wscpool = ctx.enter_context(tc.tile_pool(name="wscpool", bufs=wsb))
        psum = ctx.enter_context(tc.tile_pool(name="psum", bufs=psb, space="PSUM"))
        wqpool = ctx.enter_context(tc.tile_pool(name="wqpool", bufs=wqb, space="PSUM"))
        wpsum = ctx.enter_context(tc.tile_pool(name="wpsum", bufs=1, space="PSUM"))

        with tc.high_priority():
            ones = warmpool.tile([128, 1], f32)
            nc.vector.memset(ones, ONES_VAL)
            wz = warmpool.tile([128, 1], f32)
            nc.vector.memset(wz, 0.0)
            wbig = warmpool.tile([128, 1], f32)
            nc.vector.memset(wbig, -88.0)
            wz2 = warmpool.tile([128, 128], f32)
            nc.vector.memset(wz2, 0.0)
            wps = wpsum.tile([128, 128], f32)
            # dummy matmul chain keeps the PE p-state ramp warm from t~0.5us
            for _ in range(warm_n):
                nc.tensor.matmul(
                    out=wps[0:1, :], lhsT=wz, rhs=wz2, start=True, stop=True
                )
        wzero = warmpool.tile([128, 1], f32)

        if taper == 0:
            sizes = [16, 16] + [24] * ((bpc - 32) // 24)
        elif taper == 1:
            sizes = [16, 16] + [24] * 19 + [16, 8]
        elif taper == 2:
            sizes = [16, 16] + [24] * 19 + [8, 8, 8]
        elif taper == 3:
            sizes = [16, 16] + [24] * 18 + [16, 16, 16]
        elif taper == 4:
            sizes = [8, 16] + [24] * 20 + [8]
        elif taper == 5:
            sizes = [8, 8, 16] + [24] * 20
        elif taper == 6:
            sizes = [8, 16, 24] + [24] * 19 + [8]
        assert sum(sizes) == bpc
        pending_store = None
        b0 = 0
        gctr = 0
        for c, E in enumerate(sizes):
            G, Q = E // 8, E // 2
            fi_t = fipool.tile([128, E, N], f32)
            if c == 0:
                # warmup: split the first load across both queues
                h = E // 2
                for k in range(2):
                    eng = nc.sync if k == 0 else nc.gpsimd
                    eng.dma_start(
                        out=fi_t[:, h * k : h * k + h, :],
                        in_=fi[:, b0 + h * k : b0 + h * k + h, :],
                    )
            else:
                fi_eng = nc.sync if c % 2 == 0 else nc.gpsimd
                fi_eng.dma_start(out=fi_t, in_=fi[:, b0 : b0 + E, :])
            # d: contiguous load, then square into (w_even, 0, w_odd) slots
            d_c = dcpool.tile([128, E, S], f32)
            nc.scalar.dma_start(out=d_c, in_=dav[:, b0 : b0 + E, :])
            d_t = dpool.tile([128, Q, 3, S], f32)
            nc.vector.memset(d_t[:, :, 1, :], 0.0)
            wslots = d_t[:, :, 0:3:2, :]
            d_cv = d_c.rearrange("v (q t) s -> v q t s", t=2)
            if c == 0:
                # warm op: loads the act table while chunk 0's DMAs land and
                # produces the exact-zero bias column used by chunk 0's exp
                nc.scalar.activation(
                    wzero, wz, mybir.ActivationFunctionType.Exp,
                    scale=0.0, bias=wbig,
                )
                # per-group w pipeline so group 0's matmuls start earlier
                hq = Q // 2
                for hh in range(2):
                    wsl = wslots[:, hh * hq : (hh + 1) * hq]
                    dsl = d_cv[:, hh * hq : (hh + 1) * hq]
                    nc.vector.tensor_mul(wsl, dsl, dsl)
                    nc.scalar.activation(
                        wsl, wsl, mybir.ActivationFunctionType.Exp,
                        scale=-1.0, bias=wzero,
                    )
            else:
                nc.vector.tensor_mul(wslots, d_cv, d_cv)
                nc.scalar.activation(
                    wslots, wslots, mybir.ActivationFunctionType.Exp, scale=-1.0
                )

            if pending_store is not None:
                st_dst, st_src = pending_store
                nc.scalar.dma_start(out=st_dst, in_=st_src)

            o_t = opool.tile([128, G, N], f32)
            for g in range(G):
                ps = psum.tile([128, N], f32)
                wq = wqpool.tile([128, 1], f32)
                for jj in range(8):
                    e = g * 8 + jj
                    q, t = e // 2, e % 2
                    nc.tensor.matmul(
                        out=ps[32 * (jj // 2) : 32 * (jj // 2) + 32, :],
                        lhsT=d_t[:, q, t : t + 2, :],
                        rhs=fi_t[:, e, :],
                        start=(t == 0),
                        stop=(t == 1),
                        tile_position=(0, 32 * (jj // 2)),
                    )
                # wbar column in its own bank. HW weights APs must collapse
                # to one free dim, so per pair q: (even,zero) then (zero,odd)
                # slot windows accumulate into partitions 32q..32q+32.
                for qq in range(4):
                    q = 4 * g + qq
                    for tt in range(2):
                        nc.tensor.matmul(
                            out=wq[32 * qq : 32 * qq + 32, :],
                            lhsT=d_t[:, q, tt : tt + 2, :],
                            rhs=ones,
                            start=(tt == 0),
                            stop=(tt == 1),
                            tile_position=(0, 32 * qq),
                        )
                wsc = wscpool.tile([128, 1], f32)
                nc.vector.tensor_copy(wsc, wq)
                nc.vector.tensor_scalar_mul(o_t[:, g, :], ps, wsc)

            if c < len(sizes) - 1:
                dst = out[b0 : b0 + E].rearrange(
                    "(g jj) (s n) -> (jj s) g n", jj=8, s=S
                )
                pending_store = (dst, o_t)
            else:
                # drain fast: store the last chunk per group
                emap = {"a": [nc.scalar], "s": [nc.sync],
                        "as": [nc.scalar, nc.sync], "asg": [nc.scalar, nc.sync, nc.gpsimd]}
                engs = emap[last_eng]
                for g in range(G):
                    dst_g = out[b0 + 8 * g : b0 + 8 * g + 8].rearrange(
                        "jj (s n) -> (jj s) n", s=S
                    )
                    engs[g % len(engs)].dma_start(out=dst_g, in_=o_t[:, g, :])
            b0 += E

    return nc


_NC_CACHE = {}


def _get_nc():
    if "nc" not in _NC_CACHE:
        nc = build()
        split_multi_waits(nc)
        _NC_CACHE["nc"] = nc
    return _NC_CACHE["nc"]


def _host_pre(fi_v, d_av, core):
    sl = slice(core * BPC, (core + 1) * BPC)
    return {
        "fi_t": np.ascontiguousarray(fi_v[sl].transpose(1, 0, 2)),
        "d_t": np.ascontiguousarray(d_av[sl].transpose(1, 0, 2)),
    }


def kernel(fi_v: np.ndarray, d_av: np.ndarray) -> np.ndarray:
    from concourse.bass_utils import run_bass_kernel_spmd

    fi_v = np.ascontiguousarray(np.asarray(fi_v, dtype=np.float32))
    d_av = np.ascontiguousarray(np.asarray(d_av, dtype=np.float32))
    assert fi_v.shape == (B, V, N) and d_av.shape == (B, V, S)
    nc = _get_nc()
    in_maps = [_host_pre(fi_v, d_av, c) for c in range(NCORES)]
    res = run_bass_kernel_spmd(nc, in_maps, core_ids=list(range(NCORES)))
    return np.concatenate([res.results[c]["out"] for c in range(NCORES)], axis=0)
